# revision 3
# baseline (speedup 1.0000x reference)
"""Trainium2 Bass kernel for the attention-scoring module:

    energy   = enc @ W.T + b           # [B,S,H]
    scores   = einsum('bh,bsh->bs', hidden, energy)
    out      = softmax(scores, axis=-1)[:, None, :]

Algebraic fusion: scores[b,s] = (hidden[b] @ W) . enc[b,s] + hidden[b].b,
and the bias term is constant per row so it cancels in the softmax.  The
kernel therefore only streams enc once (memory bound), computing
v[b] = hidden[b] @ W on-device first.

Sharding: data-parallel over batch; 16 batches / 8 cores = 2 per core.
W is replicated; hidden is passed pre-transposed ([H, 2] per core).

Self-contained: hardcodes all shapes; only imports concourse/numpy.
"""

import numpy as np

B, S, H = 16, 4096, 1024
NCORES = 8
BPC = B // NCORES  # batches per core = 2
P = 128            # partitions
HC = H // P        # 8 contraction chunks for v = hidden @ W
T = 4              # 128-row blocks per enc DMA tile
SCHUNK = T * P     # 512 seq rows per DMA tile
NJ = S // SCHUNK   # 8 DMA tiles per batch
NCOL = NJ * T      # 32 score columns per batch (s = col*128 + p)

_PROGRAM = None


def _build_program():
    import concourse.bacc as bacc
    import concourse.bass_isa as bass_isa
    import concourse.mybir as mybir
    import concourse.tile as tile
    from concourse.masks import make_identity

    f32 = mybir.dt.float32
    nc = bacc.Bacc("TRN2", target_bir_lowering=False, debug=False)

    enc_d = nc.dram_tensor("enc", [BPC, S, H], f32, kind="ExternalInput").ap()
    hT_d = nc.dram_tensor("hT", [H, BPC], f32, kind="ExternalInput").ap()
    w_d = nc.dram_tensor("W", [H, H], f32, kind="ExternalInput").ap()
    out_d = nc.dram_tensor("out", [BPC, S], f32, kind="ExternalOutput").ap()

    with tile.TileContext(nc) as tc:
        with (
            tc.tile_pool(name="singles", bufs=1) as singles,
            tc.tile_pool(name="encp", bufs=4) as encp,
            tc.tile_pool(name="prodp", bufs=3) as prodp,
            tc.tile_pool(name="smallp", bufs=2) as smallp,
            tc.tile_pool(name="vpsum", bufs=2, space="PSUM") as vpsum,
            tc.tile_pool(name="tpsum", bufs=2, space="PSUM") as tpsum,
        ):
            # ---- constants / phase-0 inputs ----
            w_sb = singles.tile([P, HC, H], f32)
            for c in range(HC):
                nc.sync.dma_start(out=w_sb[:, c, :], in_=w_d[c * P:(c + 1) * P, :])
            hT_sb = singles.tile([P, HC, BPC], f32)
            nc.sync.dma_start(
                out=hT_sb, in_=hT_d.rearrange("(c p) b -> p c b", p=P)
            )
            ones = singles.tile([P, P], f32)
            nc.vector.memset(ones, 1.0)
            ident = singles.tile([P, P], f32)
            make_identity(nc, ident)

            # ---- phase 0: v[b] = hidden[b] @ W, replicated on all partitions
            # prod[g,h] = W[g,h] * hidden[b,g]  (ACT, per-partition scale),
            # then ones.T @ prod sums over g on the PE -> v_rep [128, H].
            v_sb = singles.tile([P, BPC, H], f32)
            v_ps = []
            for b in range(BPC):
                v_ps.append(
                    vpsum.tile([P, H], f32, tag="v_ps", name=f"v_ps{b}")
                )
            for c in range(HC):
                for b in range(BPC):
                    prod = prodp.tile([P, H], f32)
                    nc.scalar.mul(
                        out=prod,
                        in_=w_sb[:, c, :],
                        mul=hT_sb[:, c, b:b + 1],
                    )
                    for n in range(H // 512):
                        nc.tensor.matmul(
                            v_ps[b][:, n * 512:(n + 1) * 512],
                            ones,
                            prod[:, n * 512:(n + 1) * 512],
                            start=(c == 0),
                            stop=(c == HC - 1),
                        )
            for b in range(BPC):
                nc.scalar.copy(v_sb[:, b, :], v_ps[b])

            # ---- phase 1+2: stream enc, fused dot + softmax per batch ----
            scores = singles.tile([P, BPC, NCOL], f32)
            junk = singles.tile([P, H], f32)
            for b in range(BPC):
                for j in range(NJ):
                    et = encp.tile([P, T, H], f32)
                    nc.sync.dma_start(
                        out=et,
                        in_=enc_d[b, j * SCHUNK:(j + 1) * SCHUNK, :].rearrange(
                            "(t p) h -> p t h", p=P
                        ),
                    )
                    for t in range(T):
                        col = j * T + t
                        # fused (enc * v) + row-sum in one DVE pass
                        # (tensor_tensor_reduce crashes the exec unit on
                        # this runtime; the AFFINE_MUL_REDUCE custom-DVE op
                        # works)
                        nc.vector.affine_mul_reduce(
                            out=junk,
                            accum_out=scores[:, b, col:col + 1],
                            in0=et[:, t, :],
                            in1=v_sb[:, b, :],
                            scale=1.0,
                            bias=0.0,
                        )

                # softmax over the 4096 entries of batch b ([128, 32] layout)
                rmax = smallp.tile([P, 1], f32)
                nc.vector.tensor_reduce(
                    out=rmax, in_=scores[:, b, :],
                    axis=mybir.AxisListType.X, op=mybir.AluOpType.max,
                )
                gmax = smallp.tile([P, 1], f32)
                nc.gpsimd.partition_all_reduce(
                    gmax, rmax, channels=P, reduce_op=bass_isa.ReduceOp.max
                )
                negm = smallp.tile([P, 1], f32)
                nc.scalar.mul(out=negm, in_=gmax, mul=-1.0)
                probs = smallp.tile([P, NCOL], f32)
                sume = smallp.tile([P, 1], f32)
                nc.scalar.activation(
                    out=probs,
                    in_=scores[:, b, :],
                    func=mybir.ActivationFunctionType.Exp,
                    bias=negm,
                    scale=1.0,
                    accum_out=sume,
                )
                gsum = smallp.tile([P, 1], f32)
                nc.gpsimd.partition_all_reduce(
                    gsum, sume, channels=P, reduce_op=bass_isa.ReduceOp.add
                )
                rinv = smallp.tile([P, 1], f32)
                nc.vector.reciprocal(rinv, gsum)
                pn = smallp.tile([P, NCOL], f32)
                nc.vector.tensor_scalar_mul(out=pn, in0=probs, scalar1=rinv)

                # transpose [128, 32] -> [32, 128] so the output DMA writes
                # 512B-contiguous runs (s = col*128 + p).
                pt_ps = tpsum.tile([NCOL, P], f32)
                nc.tensor.transpose(pt_ps, pn, ident)
                pt = smallp.tile([NCOL, P], f32)
                nc.scalar.copy(pt, pt_ps)
                nc.sync.dma_start(
                    out=out_d[b].rearrange("(c p) -> c p", p=P), in_=pt
                )

    nc.compile()
    return nc


def _get_program():
    global _PROGRAM
    if _PROGRAM is None:
        _PROGRAM = _build_program()
    return _PROGRAM


def make_in_maps(hidden, encoder_outputs, W):
    hidden = np.asarray(hidden, dtype=np.float32)
    encoder_outputs = np.asarray(encoder_outputs, dtype=np.float32)
    W = np.ascontiguousarray(np.asarray(W, dtype=np.float32))
    in_maps = []
    for r in range(NCORES):
        sl = slice(BPC * r, BPC * (r + 1))
        in_maps.append({
            "enc": np.ascontiguousarray(encoder_outputs[sl]),
            "hT": np.ascontiguousarray(hidden[sl].T),
            "W": W,
        })
    return in_maps


def kernel(hidden, encoder_outputs, W, b):
    """Full-input entry point. `b` provably cancels in the softmax (it only
    adds a per-row constant to the scores) and is unused."""
    from concourse.bass_utils import run_bass_kernel_spmd

    nc = _get_program()
    in_maps = make_in_maps(hidden, encoder_outputs, W)
    res = run_bass_kernel_spmd(nc, in_maps, core_ids=list(range(NCORES)))
    out = np.concatenate([r["out"] for r in res.results], axis=0)  # [16, 4096]
    return out.reshape(B, 1, S).astype(np.float32)


# revision 4
# speedup vs baseline: 1.0099x; 1.0099x over previous
"""Trainium2 Bass kernel for the attention-scoring module:

    energy   = enc @ W.T + b           # [B,S,H]
    scores   = einsum('bh,bsh->bs', hidden, energy)
    out      = softmax(scores, axis=-1)[:, None, :]

Algebraic fusion: scores[b,s] = (hidden[b] @ W) . enc[b,s] + hidden[b].b,
and the bias term is constant per row so it cancels in the softmax.  The
kernel therefore only streams enc once (memory bound), computing
v[b] = hidden[b] @ W on-device first (ACT per-partition scale + PE
ones-matmul partition reduction, fully off the Vector engine which is the
streaming bottleneck).

Sharding: data-parallel over batch; 16 batches / 8 cores = 2 per core.
W is replicated; hidden is passed pre-shuffled as hTr[p, c*2+b] =
hidden[b, c*128+p] so the on-device layout needs one tiny contiguous DMA.

Self-contained: hardcodes all shapes; only imports concourse/numpy.
"""

import numpy as np

B, S, H = 16, 4096, 1024
NCORES = 8
BPC = B // NCORES  # batches per core = 2
P = 128            # partitions
HC = H // P        # 8 contraction chunks for v = hidden @ W
T = 4              # 128-row blocks per enc DMA tile
SCHUNK = T * P     # 512 seq rows per DMA tile
NJ = S // SCHUNK   # 8 DMA tiles per batch
NCOL = NJ * T      # 32 score columns per batch (s = col*128 + p)

_PROGRAM = None


def _build_program():
    import concourse.bacc as bacc
    import concourse.bass_isa as bass_isa
    import concourse.mybir as mybir
    import concourse.tile as tile
    from concourse.masks import make_identity

    f32 = mybir.dt.float32
    nc = bacc.Bacc("TRN2", target_bir_lowering=False, debug=False)

    enc_d = nc.dram_tensor("enc", [BPC, S, H], f32, kind="ExternalInput").ap()
    hTr_d = nc.dram_tensor("hTr", [P, HC * BPC], f32, kind="ExternalInput").ap()
    w_d = nc.dram_tensor("W", [H, H], f32, kind="ExternalInput").ap()
    out_d = nc.dram_tensor("out", [BPC, S], f32, kind="ExternalOutput").ap()

    with tile.TileContext(nc) as tc:
        with (
            tc.tile_pool(name="singles", bufs=1) as singles,
            tc.tile_pool(name="encp", bufs=4) as encp,
            tc.tile_pool(name="prodp", bufs=3) as prodp,
            tc.tile_pool(name="smallp", bufs=2) as smallp,
            tc.tile_pool(name="vpsum", bufs=2, space="PSUM") as vpsum,
            tc.tile_pool(name="tpsum", bufs=2, space="PSUM") as tpsum,
        ):
            # ---- inputs, in DMA priority order (single FIFO queue):
            # hTr first (tiny), then W chunks (gate phase 0), then enc tiles.
            hTr_sb = singles.tile([P, HC * BPC], f32)
            nc.sync.dma_start(out=hTr_sb, in_=hTr_d)
            w_sb = singles.tile([P, HC, H], f32)
            for c in range(HC):
                nc.sync.dma_start(out=w_sb[:, c, :], in_=w_d[c * P:(c + 1) * P, :])

            # enc DMAs issued now so they queue right behind W; the compute
            # below only references tiles, so Tile's scheduler keeps this
            # order on the sync engine.
            enc_tiles = {}
            for b in range(BPC):
                for j in range(NJ):
                    et = encp.tile([P, T, H], f32, name=f"et{b}_{j}", tag="et")
                    nc.sync.dma_start(
                        out=et,
                        in_=enc_d[b, j * SCHUNK:(j + 1) * SCHUNK, :].rearrange(
                            "(t p) h -> p t h", p=P
                        ),
                    )
                    enc_tiles[(b, j)] = et

            ones = singles.tile([P, P], f32)
            nc.vector.memset(ones, 1.0)
            ident = singles.tile([P, P], f32)
            make_identity(nc, ident)

            # ---- phase 0: v[b] = hidden[b] @ W, replicated on all partitions
            # prod[g,h] = W[g,h] * hidden[b,g]  (ACT, per-partition scale),
            # then ones.T @ prod sums over g on the PE -> v_rep [128, H].
            # Batch-outer so v[0] is ready as early as possible.
            v_sb = singles.tile([P, BPC, H], f32)
            for b in range(BPC):
                v_ps = vpsum.tile([P, H], f32, tag="v_ps", name=f"v_ps{b}")
                for c in range(HC):
                    prod = prodp.tile([P, H], f32)
                    nc.scalar.mul(
                        out=prod,
                        in_=w_sb[:, c, :],
                        mul=hTr_sb[:, c * BPC + b:c * BPC + b + 1],
                    )
                    for n in range(H // 512):
                        nc.tensor.matmul(
                            v_ps[:, n * 512:(n + 1) * 512],
                            ones,
                            prod[:, n * 512:(n + 1) * 512],
                            start=(c == 0),
                            stop=(c == HC - 1),
                        )
                nc.scalar.copy(v_sb[:, b, :], v_ps)

            # ---- phase 1+2: stream enc, fused dot + softmax per batch ----
            scores = singles.tile([P, BPC, NCOL], f32)
            junk = singles.tile([P, H], f32)
            for b in range(BPC):
                for j in range(NJ):
                    et = enc_tiles[(b, j)]
                    for t in range(T):
                        col = j * T + t
                        # fused (enc * v) + row-sum in one DVE pass
                        # (tensor_tensor_reduce crashes the exec unit on
                        # this runtime; the AFFINE_MUL_REDUCE custom-DVE op
                        # works)
                        nc.vector.affine_mul_reduce(
                            out=junk,
                            accum_out=scores[:, b, col:col + 1],
                            in0=et[:, t, :],
                            in1=v_sb[:, b, :],
                            scale=1.0,
                            bias=0.0,
                        )

                # softmax over the 4096 entries of batch b ([128, 32] layout)
                rmax = smallp.tile([P, 1], f32)
                nc.vector.tensor_reduce(
                    out=rmax, in_=scores[:, b, :],
                    axis=mybir.AxisListType.X, op=mybir.AluOpType.max,
                )
                gmax = smallp.tile([P, 1], f32)
                nc.gpsimd.partition_all_reduce(
                    gmax, rmax, channels=P, reduce_op=bass_isa.ReduceOp.max
                )
                negm = smallp.tile([P, 1], f32)
                nc.scalar.mul(out=negm, in_=gmax, mul=-1.0)
                probs = smallp.tile([P, NCOL], f32)
                sume = smallp.tile([P, 1], f32)
                nc.scalar.activation(
                    out=probs,
                    in_=scores[:, b, :],
                    func=mybir.ActivationFunctionType.Exp,
                    bias=negm,
                    scale=1.0,
                    accum_out=sume,
                )
                gsum = smallp.tile([P, 1], f32)
                nc.gpsimd.partition_all_reduce(
                    gsum, sume, channels=P, reduce_op=bass_isa.ReduceOp.add
                )
                rinv = smallp.tile([P, 1], f32)
                nc.vector.reciprocal(rinv, gsum)
                pn = smallp.tile([P, NCOL], f32)
                nc.vector.tensor_scalar_mul(out=pn, in0=probs, scalar1=rinv)

                # transpose [128, 32] -> [32, 128] so the output DMA writes
                # 512B-contiguous runs (s = col*128 + p).
                pt_ps = tpsum.tile([NCOL, P], f32)
                nc.tensor.transpose(pt_ps, pn, ident)
                pt = smallp.tile([NCOL, P], f32)
                nc.scalar.copy(pt, pt_ps)
                nc.sync.dma_start(
                    out=out_d[b].rearrange("(c p) -> c p", p=P), in_=pt
                )

    nc.compile()
    return nc


def _get_program():
    global _PROGRAM
    if _PROGRAM is None:
        _PROGRAM = _build_program()
    return _PROGRAM


def make_in_maps(hidden, encoder_outputs, W):
    hidden = np.asarray(hidden, dtype=np.float32)
    encoder_outputs = np.asarray(encoder_outputs, dtype=np.float32)
    W = np.ascontiguousarray(np.asarray(W, dtype=np.float32))
    in_maps = []
    for r in range(NCORES):
        sl = slice(BPC * r, BPC * (r + 1))
        hshard = hidden[sl]  # [BPC, H]
        # hTr[p, c*BPC+b] = hidden[b, c*128+p]
        hTr = np.ascontiguousarray(
            hshard.reshape(BPC, HC, P).transpose(2, 1, 0).reshape(P, HC * BPC)
        )
        in_maps.append({
            "enc": np.ascontiguousarray(encoder_outputs[sl]),
            "hTr": hTr,
            "W": W,
        })
    return in_maps


def kernel(hidden, encoder_outputs, W, b):
    """Full-input entry point. `b` provably cancels in the softmax (it only
    adds a per-row constant to the scores) and is unused."""
    from concourse.bass_utils import run_bass_kernel_spmd

    nc = _get_program()
    in_maps = make_in_maps(hidden, encoder_outputs, W)
    res = run_bass_kernel_spmd(nc, in_maps, core_ids=list(range(NCORES)))
    out = np.concatenate([r["out"] for r in res.results], axis=0)  # [16, 4096]
    return out.reshape(B, 1, S).astype(np.float32)


# revision 8
# speedup vs baseline: 1.1666x; 1.1552x over previous
"""Trainium2 Bass kernel for the attention-scoring module:

    energy   = enc @ W.T + b           # [B,S,H]
    scores   = einsum('bh,bsh->bs', hidden, energy)
    out      = softmax(scores, axis=-1)[:, None, :]

Algebraic fusion: scores[b,s] = (hidden[b] @ W) . enc[b,s] + hidden[b].b,
and the bias term is constant per row so it cancels in the softmax.  The
kernel therefore only streams enc once (memory bound), computing
v[b] = hidden[b] @ W on-device first (ACT per-partition scale + PE
ones-matmul partition reduction, fully off the Vector engine which is the
streaming bottleneck).

Sharding: data-parallel over batch; 16 batches / 8 cores = 2 per core.
W is replicated; hidden is passed pre-shuffled as hTr[p, c*2+b] =
hidden[b, c*128+p] so the on-device layout needs one tiny contiguous DMA.

Self-contained: hardcodes all shapes; only imports concourse/numpy.
"""

import numpy as np

B, S, H = 16, 4096, 1024
NCORES = 8
BPC = B // NCORES  # batches per core = 2
P = 128            # partitions
HC = H // P        # 8 contraction chunks for v = hidden @ W
T = 4              # 128-row blocks per enc DMA tile
SCHUNK = T * P     # 512 seq rows per DMA tile
NJ = S // SCHUNK   # 8 DMA tiles per batch
NCOL = NJ * T      # 32 score columns per batch (s = col*128 + p)

_PROGRAM = None


def _build_program():
    import concourse.bacc as bacc
    import concourse.bass_isa as bass_isa
    import concourse.mybir as mybir
    import concourse.tile as tile
    from concourse.masks import make_identity

    f32 = mybir.dt.float32
    nc = bacc.Bacc("TRN2", target_bir_lowering=False, debug=False)

    enc_d = nc.dram_tensor("enc", [BPC, S, H], f32, kind="ExternalInput").ap()
    hTr_d = nc.dram_tensor("hTr", [P, HC * BPC], f32, kind="ExternalInput").ap()
    w_d = nc.dram_tensor("W", [H, H], f32, kind="ExternalInput").ap()
    out_d = nc.dram_tensor("out", [BPC, S], f32, kind="ExternalOutput").ap()

    with tile.TileContext(nc) as tc:
        with (
            tc.tile_pool(name="singles", bufs=1) as singles,
            tc.tile_pool(name="encp", bufs=6) as encp,
            tc.tile_pool(name="prodp", bufs=3) as prodp,
            tc.tile_pool(name="smallp", bufs=2) as smallp,
            tc.tile_pool(name="vpsum", bufs=2, space="PSUM") as vpsum,
            tc.tile_pool(name="tpsum", bufs=2, space="PSUM") as tpsum,
        ):
            # ---- inputs, in DMA priority order (single FIFO queue):
            # hTr first (tiny), then W chunks (gate phase 0), then enc tiles.
            hTr_sb = singles.tile([P, HC * BPC], f32)
            nc.sync.dma_start(out=hTr_sb, in_=hTr_d)
            # W in 16 half-chunk DMAs so the first prod starts as early as
            # possible (prods are W-arrival paced).
            w_sb = singles.tile([P, HC, H], f32)
            for c in range(HC):
                for hh in range(2):
                    nc.sync.dma_start(
                        out=w_sb[:, c, hh * 512:(hh + 1) * 512],
                        in_=w_d[c * P:(c + 1) * P, hh * 512:(hh + 1) * 512],
                    )

            # enc DMAs issued now so they queue right behind W; the compute
            # below only references tiles, so Tile's scheduler keeps this
            # order on the sync engine.
            enc_tiles = {}
            for b in range(BPC):
                for j in range(NJ):
                    et = encp.tile([P, T, H], f32, name=f"et{b}_{j}", tag="et")
                    nc.sync.dma_start(
                        out=et,
                        in_=enc_d[b, j * SCHUNK:(j + 1) * SCHUNK, :].rearrange(
                            "(t p) h -> p t h", p=P
                        ),
                    )
                    enc_tiles[(b, j)] = et

            ones = singles.tile([P, P], f32)
            nc.vector.memset(ones, 1.0)
            ident = singles.tile([P, P], f32)
            make_identity(nc, ident)

            # ---- phase 0: v[b] = hidden[b] @ W, replicated on all partitions
            # prod[g,h] = W[g,h] * hidden[b,g]  (ACT, per-partition scale),
            # then ones.T @ prod sums over g on the PE -> v_rep [128, H].
            # Batch-outer so v[0] is ready as early as possible.
            v_sb = singles.tile([P, BPC, H], f32)
            for b in range(BPC):
                v_ps = vpsum.tile([P, H], f32, tag="v_ps", name=f"v_ps{b}")
                for c in range(HC):
                    for hh in range(2):
                        prod = prodp.tile([P, 512], f32)
                        nc.scalar.mul(
                            out=prod,
                            in_=w_sb[:, c, hh * 512:(hh + 1) * 512],
                            mul=hTr_sb[:, c * BPC + b:c * BPC + b + 1],
                        )
                        nc.tensor.matmul(
                            v_ps[:, hh * 512:(hh + 1) * 512],
                            ones,
                            prod,
                            start=(c == 0),
                            stop=(c == HC - 1),
                        )
                # PSUM->SBUF on the (idle) Vector engine: the ACT queue is
                # already busy with the next batch's prods.
                nc.vector.tensor_copy(v_sb[:, b, :], v_ps)

            # ---- phase 1+2: stream enc, fused dot + softmax per batch ----
            # per-batch score tiles: avoids a false WAR between batch 0's
            # softmax reads and batch 1's first accumulations
            scores_t = [
                singles.tile([P, NCOL], f32, name=f"scores{b}") for b in range(BPC)
            ]
            junk = singles.tile([P, H], f32)
            for b in range(BPC):
                scores = scores_t[b]
                for j in range(NJ):
                    et = enc_tiles[(b, j)]
                    for t in range(T):
                        col = j * T + t
                        # fused (enc * v) + row-sum in one DVE pass
                        # (tensor_tensor_reduce crashes the exec unit on
                        # this runtime; the AFFINE_MUL_REDUCE custom-DVE op
                        # works)
                        nc.vector.affine_mul_reduce(
                            out=junk,
                            accum_out=scores[:, col:col + 1],
                            in0=et[:, t, :],
                            in1=v_sb[:, b, :],
                            scale=1.0,
                            bias=0.0,
                        )

                # softmax over the 4096 entries of batch b ([128, 32] layout)
                rmax = smallp.tile([P, 1], f32)
                nc.vector.tensor_reduce(
                    out=rmax, in_=scores,
                    axis=mybir.AxisListType.X, op=mybir.AluOpType.max,
                )
                gmax = smallp.tile([P, 1], f32)
                nc.gpsimd.partition_all_reduce(
                    gmax, rmax, channels=P, reduce_op=bass_isa.ReduceOp.max
                )
                negm = smallp.tile([P, 1], f32)
                nc.scalar.mul(out=negm, in_=gmax, mul=-1.0)
                probs = smallp.tile([P, NCOL], f32)
                sume = smallp.tile([P, 1], f32)
                nc.scalar.activation(
                    out=probs,
                    in_=scores,
                    func=mybir.ActivationFunctionType.Exp,
                    bias=negm,
                    scale=1.0,
                    accum_out=sume,
                )
                gsum = smallp.tile([P, 1], f32)
                nc.gpsimd.partition_all_reduce(
                    gsum, sume, channels=P, reduce_op=bass_isa.ReduceOp.add
                )
                rinv = smallp.tile([P, 1], f32)
                nc.vector.reciprocal(rinv, gsum)
                pn = smallp.tile([P, NCOL], f32)
                nc.vector.tensor_scalar_mul(out=pn, in0=probs, scalar1=rinv)

                # transpose [128, 32] -> [32, 128] so the output DMA writes
                # 512B-contiguous runs (s = col*128 + p).
                pt_ps = tpsum.tile([NCOL, P], f32)
                nc.tensor.transpose(pt_ps, pn, ident)
                pt = smallp.tile([NCOL, P], f32)
                nc.scalar.copy(pt, pt_ps)
                nc.sync.dma_start(
                    out=out_d[b].rearrange("(c p) -> c p", p=P), in_=pt
                )

    nc.compile()
    return nc


def _get_program():
    global _PROGRAM
    if _PROGRAM is None:
        _PROGRAM = _build_program()
    return _PROGRAM


def make_in_maps(hidden, encoder_outputs, W):
    hidden = np.asarray(hidden, dtype=np.float32)
    encoder_outputs = np.asarray(encoder_outputs, dtype=np.float32)
    W = np.ascontiguousarray(np.asarray(W, dtype=np.float32))
    in_maps = []
    for r in range(NCORES):
        sl = slice(BPC * r, BPC * (r + 1))
        hshard = hidden[sl]  # [BPC, H]
        # hTr[p, c*BPC+b] = hidden[b, c*128+p]
        hTr = np.ascontiguousarray(
            hshard.reshape(BPC, HC, P).transpose(2, 1, 0).reshape(P, HC * BPC)
        )
        in_maps.append({
            "enc": np.ascontiguousarray(encoder_outputs[sl]),
            "hTr": hTr,
            "W": W,
        })
    return in_maps


def kernel(hidden, encoder_outputs, W, b):
    """Full-input entry point. `b` provably cancels in the softmax (it only
    adds a per-row constant to the scores) and is unused."""
    from concourse.bass_utils import run_bass_kernel_spmd

    nc = _get_program()
    in_maps = make_in_maps(hidden, encoder_outputs, W)
    res = run_bass_kernel_spmd(nc, in_maps, core_ids=list(range(NCORES)))
    out = np.concatenate([r["out"] for r in res.results], axis=0)  # [16, 4096]
    return out.reshape(B, 1, S).astype(np.float32)
